# revision 4
# baseline (speedup 1.0000x reference)
"""Trainium2 Bass kernel for masked causal dense attention.

Problem: B=8, Tq=Tv=2048, D=512 fp32.
  scores = q @ v^T; mask = v_mask & causal; scores -= 1e9*(~mask)
  out = softmax(scores) @ v; out *= q_mask

Sharding: data-parallel over batch, one batch element per NeuronCore (8 cores).
Per core: flash-style tiling over 128-row q blocks; causal structure skips
v-blocks above the diagonal. v_mask is applied additively (-1e9) from a
host-precomputed vector broadcast across partitions; the causal mask inside the
diagonal 128x128 block is applied with an on-device triangular additive mask.
Row max / exp / row sum use the fused DVE tensor_tensor_reduce and the scalar
engine's activation-with-accumulate. P is transposed 128x128 via the PE
transpose path, then the PV matmul accumulates in PSUM over v blocks.
"""

import os
import sys

import numpy as np

for _p in ("/opt/trn_rl_repo", "/root/.axon_site/_ro/trn_rl_repo"):
    if os.path.isdir(_p) and _p not in sys.path:
        sys.path.insert(0, _p)

import concourse.bacc as bacc
import concourse.bass as bass
import concourse.mybir as mybir
import concourse.tile as tile
from concourse.bass_utils import run_bass_kernel_spmd

B, Tq, Tv, D = 8, 2048, 2048, 512
P = 128
NB = Tq // P      # q blocks
ND = D // P       # contraction chunks for the S matmul
NVB = Tv // P     # v blocks
NEG = 1.0e9
F32 = mybir.dt.float32
F32R = mybir.dt.float32r

# Matmul dtype config: "f32" (exact, 4 cyc/row) or "f32r" (tf32-like, 1 cyc/row)
S_DTYPE = os.environ.get("ATTN_S_DTYPE", "f32r")
O_DTYPE = os.environ.get("ATTN_O_DTYPE", "f32r")


def _mm_dt(name):
    return F32R if name == "f32r" else F32


def build_nc(s_dtype=None, o_dtype=None):
    s_dt = _mm_dt(s_dtype or S_DTYPE)
    o_dt = _mm_dt(o_dtype or O_DTYPE)

    nc = bacc.Bacc("TRN2", target_bir_lowering=False, num_devices=B)
    qt = nc.dram_tensor("qt", [D, Tq], F32, kind="ExternalInput")
    vt = nc.dram_tensor("vt", [D, Tv], F32, kind="ExternalInput")
    v = nc.dram_tensor("v", [Tv, D], F32, kind="ExternalInput")
    negv = nc.dram_tensor("negv", [Tv], F32, kind="ExternalInput")
    qsc = nc.dram_tensor("qsc", [Tq], F32, kind="ExternalInput")
    out = nc.dram_tensor("out", [Tq, D], F32, kind="ExternalOutput")

    from contextlib import ExitStack

    with tile.TileContext(nc) as tc, ExitStack() as ctx:
        const = ctx.enter_context(tc.tile_pool(name="const", bufs=1))
        big = ctx.enter_context(tc.tile_pool(name="big", bufs=1))
        qtp = ctx.enter_context(tc.tile_pool(name="qtp", bufs=3))
        pp = ctx.enter_context(tc.tile_pool(name="pp", bufs=2))
        ptp = ctx.enter_context(tc.tile_pool(name="ptp", bufs=2))
        outp = ctx.enter_context(tc.tile_pool(name="outp", bufs=3))
        smallp = ctx.enter_context(tc.tile_pool(name="smallp", bufs=3))
        sps = ctx.enter_context(tc.tile_pool(name="sps", bufs=4, space="PSUM"))
        ops = ctx.enter_context(tc.tile_pool(name="ops", bufs=2, space="PSUM"))
        pts = ctx.enter_context(tc.tile_pool(name="pts", bufs=2, space="PSUM"))

        if True:
            # --- constants ---
            ident = const.tile([P, P], o_dt)
            nc.gpsimd.memset(ident, 0.0)
            nc.gpsimd.affine_select(
                out=ident, in_=ident, compare_op=mybir.AluOpType.not_equal,
                fill=1.0, base=0, pattern=[[-1, P]], channel_multiplier=1,
            )
            # tri[q, v] = -NEG where v > q else 0  (within-diagonal-block causal)
            tri = const.tile([P, P], F32)
            nc.gpsimd.memset(tri, 0.0)
            nc.gpsimd.affine_select(
                out=tri, in_=tri, compare_op=mybir.AluOpType.is_ge,
                fill=-NEG, base=0, pattern=[[-1, P]], channel_multiplier=1,
            )

            # --- persistent per-core data ---
            negv_b = big.tile([P, Tv], F32)
            nc.sync.dma_start(out=negv_b, in_=negv.ap().partition_broadcast(P))
            qsc_sb = big.tile([P, NB], F32)
            nc.sync.dma_start(
                out=qsc_sb, in_=qsc.ap().rearrange("(b p) -> p b", p=P)
            )
            # vt_sb[:, c, :] = vt[c*128:(c+1)*128, :]   (rhs of S matmul)
            vt_sb = big.tile([P, ND, Tv], s_dt)
            for c in range(ND):
                if s_dt == F32:
                    nc.sync.dma_start(
                        out=vt_sb[:, c, :], in_=vt[c * P:(c + 1) * P, :]
                    )
                else:
                    stage = qtp.tile([P, Tv], F32, tag="vstage")
                    nc.sync.dma_start(out=stage, in_=vt[c * P:(c + 1) * P, :])
                    nc.vector.tensor_copy(vt_sb[:, c, :], stage)
            # v_sb[:, j, :] = v[j*128:(j+1)*128, :]   (rhs of O matmul)
            v_sb = big.tile([P, NVB, D], o_dt)
            for j in range(NVB):
                if o_dt == F32:
                    nc.sync.dma_start(
                        out=v_sb[:, j, :], in_=v[j * P:(j + 1) * P, :]
                    )
                else:
                    stage = qtp.tile([P, D], F32, tag="vstage2")
                    nc.sync.dma_start(out=stage, in_=v[j * P:(j + 1) * P, :])
                    nc.vector.tensor_copy(v_sb[:, j, :], stage)

            def emit_softmax_block(b):
                """S matmuls + masked softmax for q block b.
                Returns (p_sb, fs, W) for the deferred transpose+PV stage."""
                nvb = b + 1               # causal: v blocks 0..b
                W = nvb * P               # active score width
                nch = (W + 511) // 512    # PSUM chunks

                qt_t = qtp.tile([P, D], F32, tag="qt")
                for c in range(ND):
                    nc.sync.dma_start(
                        out=qt_t[:, c * P:(c + 1) * P],
                        in_=qt[c * P:(c + 1) * P, b * P:(b + 1) * P],
                    )
                if s_dt == F32:
                    qt_use = qt_t
                else:
                    qt_use = qtp.tile([P, D], s_dt, tag="qtr")
                    nc.vector.tensor_copy(qt_use, qt_t)

                p_sb = pp.tile([P, W], F32 if o_dt == F32 else o_dt, tag="p")
                colmax = smallp.tile([P, 4], F32, tag="colmax")
                lsum = smallp.tile([P, 4], F32, tag="lsum")
                negm = smallp.tile([P, 1], F32, tag="negm")
                s_tiles = []
                for c in range(nch):
                    v0 = c * 512
                    w = min(512, W - v0)
                    s_t = sps.tile([P, 512], F32, tag="s")
                    s_tiles.append((s_t, v0, w))
                    for dc in range(ND):
                        nc.tensor.matmul(
                            s_t[:, :w],
                            qt_use[:, dc * P:(dc + 1) * P],
                            vt_sb[:, dc, v0:v0 + w],
                            start=(dc == 0),
                            stop=(dc == ND - 1),
                        )
                    if c == nch - 1:
                        # causal mask inside the diagonal 128-col block
                        nc.vector.tensor_add(
                            out=s_t[:, w - P:w], in0=s_t[:, w - P:w], in1=tri
                        )
                    # s += v-mask penalty; chunk row-max
                    # (fused tensor_tensor_reduce crashes the exec unit in
                    # this environment -- use separate add + reduce)
                    nc.vector.tensor_add(
                        out=s_t[:, :w], in0=s_t[:, :w],
                        in1=negv_b[:, v0:v0 + w],
                    )
                    nc.vector.reduce_max(
                        out=colmax[:, c:c + 1], in_=s_t[:, :w],
                        axis=mybir.AxisListType.X,
                    )
                # negm = -max over chunks
                nc.vector.tensor_reduce(
                    out=negm, in_=colmax[:, :nch], axis=mybir.AxisListType.X,
                    op=mybir.AluOpType.max, negate=True,
                )
                for c, (s_t, v0, w) in enumerate(s_tiles):
                    nc.scalar.activation(
                        out=p_sb[:, v0:v0 + w], in_=s_t[:, :w],
                        func=mybir.ActivationFunctionType.Exp,
                        bias=negm, scale=1.0,
                        accum_out=lsum[:, c:c + 1],
                    )
                l = smallp.tile([P, 1], F32, tag="l")
                nc.vector.tensor_reduce(
                    out=l, in_=lsum[:, :nch], axis=mybir.AxisListType.X,
                    op=mybir.AluOpType.add,
                )
                linv = smallp.tile([P, 1], F32, tag="linv")
                nc.vector.reciprocal(out=linv, in_=l)
                fs = smallp.tile([P, 1], F32, tag="fs")
                nc.vector.tensor_mul(fs, linv, qsc_sb[:, b:b + 1])
                return p_sb, fs, W

            def emit_pv_block(b, p_sb, fs, W):
                """Transpose P and accumulate O = P^T.T @ V for q block b."""
                nvb = W // P
                pt_sb = ptp.tile([P, W], o_dt, tag="pt")
                for g in range(0, nvb, 4):
                    gn = min(4, nvb - g)
                    pt_ps = pts.tile([P, 512], o_dt, tag="ptps")
                    for k in range(gn):
                        j = g + k
                        nc.tensor.transpose(
                            out=pt_ps[:, k * P:(k + 1) * P],
                            in_=p_sb[:, j * P:(j + 1) * P],
                            identity=ident,
                        )
                    nc.vector.tensor_copy(
                        pt_sb[:, g * P:(g + gn) * P], pt_ps[:, :gn * P]
                    )
                o_ps = ops.tile([P, D], F32, tag="o")
                for j in range(nvb):
                    nc.tensor.matmul(
                        o_ps,
                        pt_sb[:, j * P:(j + 1) * P],
                        v_sb[:, j, :],
                        start=(j == 0),
                        stop=(j == nvb - 1),
                    )
                o_sb = outp.tile([P, D], F32, tag="osb")
                nc.vector.tensor_scalar_mul(out=o_sb, in0=o_ps, scalar1=fs)
                nc.sync.dma_start(out=out[b * P:(b + 1) * P, :], in_=o_sb)

            # software pipeline: S/softmax of block b overlaps PV of block b-1
            prev = None
            for b in range(NB):
                cur = emit_softmax_block(b)
                if prev is not None:
                    emit_pv_block(prev[0], *prev[1])
                prev = (b, cur)
            emit_pv_block(prev[0], *prev[1])

    nc.compile()
    return nc


_NC_CACHE = {}


def _get_nc():
    key = (S_DTYPE, O_DTYPE)
    if key not in _NC_CACHE:
        _NC_CACHE[key] = build_nc()
    return _NC_CACHE[key]


def make_in_maps(query, value, q_mask, v_mask):
    in_maps = []
    for b in range(B):
        q = np.asarray(query[b], dtype=np.float32)
        val = np.asarray(value[b], dtype=np.float32)
        in_maps.append({
            "qt": np.ascontiguousarray(q.T),
            "vt": np.ascontiguousarray(val.T),
            "v": np.ascontiguousarray(val),
            "negv": np.where(v_mask[b], 0.0, -NEG).astype(np.float32),
            "qsc": np.asarray(q_mask[b], dtype=np.float32),
        })
    return in_maps


def kernel(query, value, q_mask, v_mask, **kw):
    nc = _get_nc()
    in_maps = make_in_maps(query, value, q_mask, v_mask)
    res = run_bass_kernel_spmd(nc, in_maps, core_ids=list(range(B)))
    return np.stack([res.results[c]["out"] for c in range(B)], axis=0)
